# revision 1
# baseline (speedup 1.0000x reference)
"""Trainium2 Bass kernel: aspect-level sentiment classification head.

  aspect[b] = mean(last_hidden_state[b, start_b:end_b, :])   (ragged spans)
  out = concat([pooled, aspect], -1) @ W.T + b

Strategy: data-parallel over batch (8 samples per core, 8 cores).  The key
observation is that only the span rows of last_hidden_state are ever needed,
so each core *gathers* just those rows from DRAM with an indirect DMA whose
row indices are computed on-device from position_indices.  Spans are padded
to L = 32*m rows (m = power of two chosen from the max span length at call
time); rows past the span end are masked to zero.  The per-sample 1/len is
folded into the mask so a single PE matmul per 128-column chunk produces the
*transposed* aspect features directly, which then feed an accumulated
12-chunk GEMM against host-pre-transposed W.
"""

import os
import sys

if "/opt/trn_rl_repo" not in sys.path:
    sys.path.insert(0, "/opt/trn_rl_repo")

import numpy as np

import concourse.bass as bass
import concourse.tile as tile
from concourse import bacc, mybir
from concourse.bass import IndirectOffsetOnAxis
from concourse.bass_utils import run_bass_kernel_spmd

F32 = mybir.dt.float32
I32 = mybir.dt.int32

B, S, H, C = 64, 4096, 768, 3
NCORES = 8
BL = B // NCORES          # samples per core
P = 128
HC = H // P               # 6 hidden chunks of 128
KC = 2 * H // P           # 12 contraction chunks in the final GEMM


def _log2(x: int) -> int:
    l = x.bit_length() - 1
    assert 1 << l == x
    return l


def build(m: int):
    """Build + compile the per-core SPMD program for spans up to 32*m rows."""
    assert m & (m - 1) == 0 and 1 <= m <= S // 32
    nblk = BL * m            # 32-row blocks per core
    G = nblk // 4            # gather groups of 128 rows
    cols = max(1, 4 // m)    # samples covered by one group
    gps = max(1, m // 4)     # groups per sample
    lm = _log2(m)

    nc = bacc.Bacc("TRN2", target_bir_lowering=False, debug=False,
                   num_devices=NCORES)
    lhs = nc.dram_tensor("lhs", [BL * S, H], F32, kind="ExternalInput").ap()
    # packed params: pooled_r at cols 0-127, w_r at 128-255, bias row at
    # 256-258 -- one DMA instead of three
    blob = nc.dram_tensor("blob", [HC * BL, 2 * P + C], F32,
                          kind="ExternalInput").ap()
    pos = nc.dram_tensor("pos", [BL, 2], I32, kind="ExternalInput").ap()
    out = nc.dram_tensor("out", [BL, C], F32, kind="ExternalOutput").ap()

    with tile.TileContext(nc) as tc:
        packed = m <= 4  # one PSUM bank for all 6 aspect chunks vs 6 banks
        with (
            tc.tile_pool(name="const", bufs=1) as cp,
            tc.tile_pool(name="work", bufs=4) as wp,
            tc.tile_pool(name="rows", bufs=4) as rp,
            tc.tile_pool(name="pmisc", bufs=1, space="PSUM") as pm,
            tc.tile_pool(name="pbc", bufs=2 if packed else 1,
                         space="PSUM") as pb,
            tc.tile_pool(name="pasp", bufs=1, space="PSUM") as pa,
        ):
            # ---- constants / params -------------------------------------
            id48 = cp.tile([HC * BL, HC * BL], F32, tag="id48")
            from concourse.masks import make_identity
            make_identity(nc, id48[:])

            pos_i = cp.tile([BL, 2], I32, tag="pos_i")
            nc.sync.dma_start(pos_i[:], pos[:, :], single_packet=True)
            pos_f = cp.tile([BL, 2], F32, tag="pos_f")
            nc.vector.tensor_copy(pos_f[:], pos_i[:])

            blob_sb = cp.tile([HC * BL, 2 * P + C], F32, tag="blob_sb")
            nc.sync.dma_start(blob_sb[:], blob[:, :])
            bias_row = blob_sb[0:1, 2 * P:2 * P + C]
            ones18 = cp.tile([1, BL], F32, tag="ones18")
            nc.gpsimd.memset(ones18[:], 1.0)

            # transpose pooled_r -> pT [128, 48] (pT[h, c*8+b] = pooled[b, c*128+h])
            pT_ps = pm.tile([P, HC * BL], F32, tag="pmisc", name="pT_ps")
            nc.tensor.transpose(pT_ps[:], blob_sb[:, 0:P], id48[:])
            pT = cp.tile([P, HC * BL], F32, tag="pT")
            nc.vector.tensor_copy(pT[:], pT_ps[:])

            # transpose w_r -> wT [128, 36] (wT[h, c*3+j] = W[j, c*128+h])
            wT_ps = pm.tile([P, KC * C], F32, tag="pmisc", name="wT_ps")
            nc.tensor.transpose(wT_ps[:], blob_sb[0:KC * C, P:2 * P],
                                id48[: KC * C, : KC * C])
            wT = cp.tile([P, KC * C], F32, tag="wT")
            nc.vector.tensor_copy(wT[:], wT_ps[:])

            # per-partition index helpers (p = partition id, u = p >> 5)
            iota_p = cp.tile([P, 1], I32, tag="iota_p")
            nc.gpsimd.iota(iota_p[:], pattern=[[1, 1]], base=0,
                           channel_multiplier=1)
            u_i = cp.tile([P, 1], I32, tag="u_i")
            nc.vector.tensor_scalar(u_i[:], iota_p[:], 5, None,
                                    mybir.AluOpType.arith_shift_right)
            pm32_i = cp.tile([P, 1], I32, tag="pm32_i")
            nc.vector.tensor_scalar(pm32_i[:], iota_p[:], 31, None,
                                    mybir.AluOpType.bitwise_and)
            pm32_f = cp.tile([P, 1], F32, tag="pm32_f")
            nc.vector.tensor_copy(pm32_f[:], pm32_i[:])
            u_f = cp.tile([P, 1], F32, tag="u_f")
            nc.vector.tensor_copy(u_f[:], u_i[:])

            # psum accumulators for transposed aspect features; for m >= 8
            # accumulation groups stay open across gather groups, so each
            # hidden chunk needs its own bank
            if packed:
                aspT_all = pa.tile([P, HC * BL], F32, tag="aspT")
                aspT_ps = [aspT_all[:, c * BL:(c + 1) * BL]
                           for c in range(HC)]
            else:
                aspT_ps = [pa.tile([P, BL], F32, tag=f"aspT{c}",
                                   name=f"aspT{c}")[:] for c in range(HC)]

            # ---- gather groups ------------------------------------------
            for g in range(G):
                # broadcast (start, end) of each partition's sample via PE:
                # ind[s, p] = 1 iff s == (4g + p//32) >> lm
                ind = wp.tile([BL, P], F32, tag="ind")
                nc.gpsimd.memset(ind[:], 1.0)
                nc.gpsimd.affine_select(
                    out=ind[:], in_=ind[:], compare_op=mybir.AluOpType.is_ge,
                    fill=0.0, base=128 * g, channel_multiplier=-32 * m,
                    pattern=[[1, P]])
                # keep where p - 32m*s + 128g <= 32m-1, negated for is_ge
                nc.gpsimd.affine_select(
                    out=ind[:], in_=ind[:], compare_op=mybir.AluOpType.is_ge,
                    fill=0.0, base=(32 * m - 1) - 128 * g,
                    channel_multiplier=32 * m, pattern=[[-1, P]])
                bc_ps = pb.tile([P, 2], F32, tag="bc")
                nc.tensor.matmul(out=bc_ps[:], lhsT=ind[:], rhs=pos_f[:],
                                 start=True, stop=True)
                bc = wp.tile([P, 2], F32, tag="bcs")
                nc.vector.tensor_copy(bc[:], bc_ps[:])
                st_f = bc[:, 0:1]
                en_f = bc[:, 1:2]

                # row-within-span and sample base offset for this group
                if m == 1:
                    jrow_f = pm32_f[:]
                    s4096_f = wp.tile([P, 1], F32, tag="s4096")
                    # (u + 4g) * 4096
                    nc.vector.tensor_scalar(
                        s4096_f[:], u_f[:], float(4 * g), 4096.0,
                        mybir.AluOpType.add, mybir.AluOpType.mult)
                else:
                    k_i = wp.tile([P, 1], I32, tag="k_i")
                    nc.vector.tensor_scalar(k_i[:], u_i[:], 4 * g, None,
                                            mybir.AluOpType.add)
                    q32_i = wp.tile([P, 1], I32, tag="q32")
                    nc.vector.tensor_scalar(
                        q32_i[:], k_i[:], m - 1, 32,
                        mybir.AluOpType.bitwise_and, mybir.AluOpType.mult)
                    jr_i = wp.tile([P, 1], I32, tag="jr_i")
                    nc.vector.tensor_add(jr_i[:], q32_i[:], pm32_i[:])
                    jrow_ft = wp.tile([P, 1], F32, tag="jrow_f")
                    nc.vector.tensor_copy(jrow_ft[:], jr_i[:])
                    jrow_f = jrow_ft[:]
                    s4_i = wp.tile([P, 1], I32, tag="s4_i")
                    nc.vector.tensor_scalar(
                        s4_i[:], k_i[:], lm, 4096,
                        mybir.AluOpType.arith_shift_right,
                        mybir.AluOpType.mult)
                    s4096_f = wp.tile([P, 1], F32, tag="s4096")
                    nc.vector.tensor_copy(s4096_f[:], s4_i[:])

                # gather row index = min(start + jrow, S-1) + 4096*s
                row_f = wp.tile([P, 1], F32, tag="row_f")
                nc.vector.tensor_add(row_f[:], st_f, jrow_f)
                idx_f = wp.tile([P, 1], F32, tag="idx_f")
                nc.vector.tensor_scalar(
                    idx_f[:], row_f[:], float(S - 1), s4096_f[:, 0:1],
                    mybir.AluOpType.min, mybir.AluOpType.add)
                idx_i = wp.tile([P, 1], I32, tag="idx_i")
                nc.vector.tensor_copy(idx_i[:], idx_f[:])

                # mask = (jrow < len) / len  (len==0 -> NaN, matches 0/0 ref)
                len_f = wp.tile([P, 1], F32, tag="len_f")
                nc.vector.tensor_sub(len_f[:], en_f, st_f)
                recip = wp.tile([P, 1], F32, tag="recip")
                nc.vector.reciprocal(recip[:], len_f[:])
                inm = wp.tile([P, 1], F32, tag="inm")
                nc.vector.tensor_tensor(out=inm[:], in0=jrow_f, in1=len_f[:],
                                        op=mybir.AluOpType.is_lt)
                inm_s = wp.tile([P, 1], F32, tag="inm_s")
                nc.vector.tensor_mul(inm_s[:], inm[:], recip[:])

                if cols == 1:
                    maskg = inm_s[:]
                else:
                    mk = wp.tile([P, cols], F32, tag="mk")
                    nc.vector.tensor_copy(mk[:], inm_s[:, 0:1].to_broadcast(
                        [P, cols]))
                    nc.gpsimd.affine_select(
                        out=mk[:], in_=mk[:],
                        compare_op=mybir.AluOpType.is_ge, fill=0.0, base=0,
                        channel_multiplier=1, pattern=[[-32 * m, cols]])
                    # keep where p - 32m*j <= 32m-1, negated for is_ge
                    nc.gpsimd.affine_select(
                        out=mk[:], in_=mk[:],
                        compare_op=mybir.AluOpType.is_ge, fill=0.0,
                        base=32 * m - 1, channel_multiplier=-1,
                        pattern=[[32 * m, cols]])
                    maskg = mk[:]

                rows_t = rp.tile([P, H], F32, tag="rows")
                nc.gpsimd.indirect_dma_start(
                    out=rows_t[:], out_offset=None, in_=lhs[:, :],
                    in_offset=IndirectOffsetOnAxis(ap=idx_i[:, 0:1], axis=0))

                # aspT[h, s] += rows[:, chunk].T @ mask
                s_lo = (4 * g) // m
                first = g % gps == 0
                last = g % gps == gps - 1
                for c in range(HC):
                    nc.tensor.matmul(
                        out=aspT_ps[c][:, s_lo:s_lo + cols],
                        lhsT=rows_t[:, c * P:(c + 1) * P], rhs=maskg,
                        start=first, stop=last)

            # ---- final GEMM: out[b, j] = sum_k featT[k, b] * wT[k, j] ----
            aspT_sb = cp.tile([P, HC * BL], F32, tag="aspT_sb")
            if packed:
                nc.vector.tensor_copy(aspT_sb[:], aspT_all[:])
            else:
                for c in range(HC):
                    nc.vector.tensor_copy(aspT_sb[:, c * BL:(c + 1) * BL],
                                          aspT_ps[c])

            out_ps = pm.tile([BL, C], F32, tag="pmisc", name="out_ps")
            for c in range(KC):
                featT = (pT[:, (c * BL):(c + 1) * BL] if c < HC
                         else aspT_sb[:, (c - HC) * BL:(c - HC + 1) * BL])
                nc.tensor.matmul(out=out_ps[:], lhsT=featT,
                                 rhs=wT[:, c * C:(c + 1) * C],
                                 start=(c == 0), stop=False)
            # bias as a rank-1 accumulation: ones[1,8].T @ bias_row[1,3]
            nc.tensor.matmul(out=out_ps[:], lhsT=ones18[:], rhs=bias_row,
                             start=False, stop=True)

            out_sb = cp.tile([BL, C], F32, tag="out_sb")
            nc.vector.tensor_copy(out_sb[:], out_ps[:])
            nc.sync.dma_start(out[:, :], out_sb[:], single_packet=True)

    nc.compile()
    return nc


_CACHE: dict[int, object] = {}


def _get(m: int):
    if m not in _CACHE:
        _CACHE[m] = build(m)
    return _CACHE[m]


def kernel(last_hidden_state, pooled_output, position_indices, W, b):
    last_hidden_state = np.ascontiguousarray(last_hidden_state,
                                             dtype=np.float32)
    pooled_output = np.ascontiguousarray(pooled_output, dtype=np.float32)
    position_indices = np.ascontiguousarray(position_indices, dtype=np.int32)
    W = np.ascontiguousarray(W, dtype=np.float32)
    b = np.ascontiguousarray(b, dtype=np.float32)

    lens = position_indices[:, 1] - position_indices[:, 0]
    maxlen = max(1, int(lens.max()))
    m = 1
    while 32 * m < maxlen:
        m *= 2
    in_maps = _make_in_maps(m, last_hidden_state, pooled_output,
                            position_indices, W, b)
    if RUN_KWARGS:
        # profiling path (test.py sets trace=True)
        res = run_bass_kernel_spmd(_get(m), in_maps,
                                   core_ids=list(range(NCORES)),
                                   **RUN_KWARGS)
        global LAST_RESULT
        LAST_RESULT = res
        results = res.results
    else:
        results = _run_fast(m, in_maps)
    return np.concatenate([results[c]["out"] for c in range(NCORES)],
                          axis=0)


# Cached-jit fast path: run_bass_kernel_spmd re-jits its PJRT wrapper on
# every call (~17s), so repeated kernel() calls would pay the full XLA +
# neuronx-cc pipeline each time.  This replicates bass2jax.run_bass_via_pjrt
# (multi-core branch) once per m and reuses the compiled executable.
_RUNNER_CACHE: dict = {}


def _get_runner(m):
    if m in _RUNNER_CACHE:
        return _RUNNER_CACHE[m]
    import jax
    from jax.experimental.shard_map import shard_map
    from jax.sharding import Mesh, PartitionSpec
    from concourse import bass2jax

    nc = _get(m)
    bass2jax.install_neuronx_cc_hook()
    assert nc.dbg_addr is None, "fast path assumes debug-free program"
    partition_name = (nc.partition_id_tensor.name
                      if nc.partition_id_tensor else None)

    in_names, out_names, out_avals = [], [], []
    for alloc in nc.m.functions[0].allocations:
        if not isinstance(alloc, mybir.MemoryLocationSet):
            continue
        name = alloc.memorylocations[0].name
        if alloc.kind == "ExternalInput":
            if name != partition_name:
                in_names.append(name)
        elif alloc.kind == "ExternalOutput":
            shape = tuple(alloc.tensor_shape)
            dtype = mybir.dt.np(alloc.dtype)
            out_names.append(name)
            out_avals.append(jax.core.ShapedArray(shape, dtype))
    n_params = len(in_names)
    n_outs = len(out_avals)
    all_names = in_names + out_names
    if partition_name is not None:
        all_names = all_names + [partition_name]

    def _body(*args):
        operands = list(args)
        if partition_name is not None:
            operands.append(bass2jax.partition_id_tensor())
        outs = bass2jax._bass_exec_p.bind(
            *operands,
            out_avals=tuple(out_avals),
            in_names=tuple(all_names),
            out_names=tuple(out_names),
            lowering_input_output_aliases=(),
            sim_require_finite=True,
            sim_require_nnan=True,
            nc=nc,
        )
        return tuple(outs)

    devices = jax.devices()[:NCORES]
    mesh = Mesh(np.asarray(devices), ("core",))
    specs = (PartitionSpec("core"),) * (n_params + n_outs)
    out_specs = (PartitionSpec("core"),) * n_outs
    sharded = jax.jit(
        shard_map(_body, mesh=mesh, in_specs=specs, out_specs=out_specs,
                  check_rep=False),
        donate_argnums=tuple(range(n_params, n_params + n_outs)),
        keep_unused=True,
    )
    runner = (sharded, in_names, out_names, out_avals, n_params)
    _RUNNER_CACHE[m] = runner
    return runner


def _run_fast(m, in_maps):
    sharded, in_names, out_names, out_avals, n_params = _get_runner(m)
    concat_in = [
        np.concatenate([np.asarray(in_maps[c][k]) for c in range(NCORES)],
                       axis=0)
        for k in in_names
    ]
    concat_zeros = [
        np.zeros((NCORES * a.shape[0], *a.shape[1:]), a.dtype)
        for a in out_avals
    ]
    out_arrs = sharded(*concat_in, *concat_zeros)
    return [
        {name: np.asarray(out_arrs[i]).reshape(NCORES, *out_avals[i].shape)[c]
         for i, name in enumerate(out_names)}
        for c in range(NCORES)
    ]


def _make_in_maps(m, last_hidden_state, pooled_output, position_indices,
                  W, b):
    w_r = W.reshape(C, KC, P).transpose(1, 0, 2).reshape(KC * C, P)
    in_maps = []
    for cid in range(NCORES):
        sl = slice(cid * BL, (cid + 1) * BL)
        bl = np.zeros((HC * BL, 2 * P + C), np.float32)
        bl[:, 0:P] = (pooled_output[sl].reshape(BL, HC, P)
                      .transpose(1, 0, 2).reshape(HC * BL, P))
        bl[0:KC * C, P:2 * P] = w_r
        bl[0, 2 * P:2 * P + C] = b
        in_maps.append({
            "lhs": last_hidden_state[sl].reshape(BL * S, H),
            "pos": position_indices[sl],
            "blob": bl,
        })
    return in_maps


# test/bench hooks (harness just calls kernel(); these stay default)
RUN_KWARGS: dict = {}
LAST_RESULT = None



# revision 7
# speedup vs baseline: 1.2324x; 1.2324x over previous
"""Trainium2 Bass kernel: aspect-level sentiment classification head.

  aspect[b] = mean(last_hidden_state[b, start_b:end_b, :])   (ragged spans)
  out = concat([pooled, aspect], -1) @ W.T + b

Strategy: data-parallel over batch with host-side load balancing.  Samples
are assigned to cores (8 per core) so the per-core total span length is
minimized; each core gathers exactly its spans' rows (tightly packed, no
per-sample padding) from DRAM with G indirect DMAs of 128 rows each.  All
index/mask arithmetic happens on the host: the kernel receives ready-made
gather indices plus a bf16 "weight mask" whose entries are 1/len placed at
(row, sample) positions, so a single bf16 matmul per 128-column hidden chunk
produces the *transposed* aspect features directly.  Pooled features and W
arrive host-pre-transposed in one bf16 blob; the pooled half of the final
GEMM is issued before the gather completes so it overlaps the DMA.
"""

import sys

if "/opt/trn_rl_repo" not in sys.path:
    sys.path.insert(0, "/opt/trn_rl_repo")

import numpy as np
import ml_dtypes

import concourse.tile as tile
from concourse import bacc, mybir
from concourse.bass import IndirectOffsetOnAxis
from concourse.bass_utils import run_bass_kernel_spmd

F32 = mybir.dt.float32
BF16 = mybir.dt.bfloat16
I32 = mybir.dt.int32

B, S, H, C = 64, 4096, 768, 3
NCORES = 8
BL = B // NCORES          # samples per core
P = 128
HC = H // P               # 6 hidden chunks of 128
KC = 2 * H // P           # 12 contraction chunks in the final GEMM

# blob column layout (all bf16): pT | wT | wmask | ones8 | bias
PT0, WT0 = 0, HC * BL                        # 0, 48
def _cols(G):
    wm0 = WT0 + KC * C                       # 84
    on0 = wm0 + BL * G
    b0 = on0 + BL
    return wm0, on0, b0, b0 + C


def build(G: int):
    """Per-core SPMD program gathering G*128 tightly packed span rows."""
    WM0, ON0, B0, BW = _cols(G)
    nc = bacc.Bacc("TRN2", target_bir_lowering=False, debug=False,
                   num_devices=NCORES)
    lhs = nc.dram_tensor("lhs", [BL * S, H], F32, kind="ExternalInput").ap()
    idx = nc.dram_tensor("idx", [P, G], I32, kind="ExternalInput").ap()
    blob = nc.dram_tensor("blob", [P, BW], BF16, kind="ExternalInput").ap()
    out = nc.dram_tensor("out", [BL, C], F32, kind="ExternalOutput").ap()

    with tile.TileContext(nc) as tc:
        with (
            tc.tile_pool(name="const", bufs=1) as cp,
            tc.tile_pool(name="rows", bufs=min(2, G)) as rp,
            tc.tile_pool(name="pout", bufs=1, space="PSUM") as po,
            tc.tile_pool(name="pasp", bufs=1, space="PSUM") as pa,
        ):
            idx_sb = cp.tile([P, G], I32, tag="idx_sb")
            nc.sync.dma_start(idx_sb[:], idx[:, :])
            blob_sb = cp.tile([P, BW], BF16, tag="blob_sb")
            nc.scalar.dma_start(blob_sb[:], blob[:, :])
            pT = blob_sb[:, PT0:PT0 + HC * BL]
            wT = blob_sb[:, WT0:WT0 + KC * C]

            # pooled half of the final GEMM runs while the gather is in
            # flight; the accumulation group stays open until the bias term
            out_ps = po.tile([BL, C], F32, tag="out_ps")
            for c in range(HC):
                nc.tensor.matmul(out=out_ps[:], lhsT=pT[:, c * BL:(c + 1) * BL],
                                 rhs=wT[:, c * C:(c + 1) * C],
                                 start=(c == 0), stop=False)

            # aspT[h, s] = sum_r rows[r, h] * wmask[r, s]
            if G == 1:
                aspT_all = pa.tile([P, HC * BL], F32, tag="aspT")
                aspT_ps = [aspT_all[:, c * BL:(c + 1) * BL] for c in range(HC)]
            else:
                aspT_ps = [pa.tile([P, BL], F32, tag=f"aspT{c}",
                                   name=f"aspT{c}")[:] for c in range(HC)]
            for g in range(G):
                rows_f = rp.tile([P, H], F32, tag="rows_f")
                nc.gpsimd.indirect_dma_start(
                    out=rows_f[:], out_offset=None, in_=lhs[:, :],
                    in_offset=IndirectOffsetOnAxis(
                        ap=idx_sb[:, g:g + 1], axis=0))
                rows_b = rp.tile([P, H], BF16, tag="rows_b")
                nc.vector.tensor_copy(rows_b[:], rows_f[:])
                wm_g = blob_sb[:, WM0 + g * BL:WM0 + (g + 1) * BL]
                for c in range(HC):
                    nc.tensor.matmul(out=aspT_ps[c][:, :],
                                     lhsT=rows_b[:, c * P:(c + 1) * P],
                                     rhs=wm_g, start=(g == 0), stop=(g == G - 1))

            aspT_sb = cp.tile([P, HC * BL], BF16, tag="aspT_sb")
            if G == 1:
                nc.vector.tensor_copy(aspT_sb[:], aspT_all[:])
            else:
                for c in range(HC):
                    nc.vector.tensor_copy(aspT_sb[:, c * BL:(c + 1) * BL],
                                          aspT_ps[c])
            for c in range(HC):
                nc.tensor.matmul(out=out_ps[:],
                                 lhsT=aspT_sb[:, c * BL:(c + 1) * BL],
                                 rhs=wT[:, (HC + c) * C:(HC + c + 1) * C],
                                 start=False, stop=False)
            # bias as a rank-1 accumulation: ones[1,8].T @ bias_row[1,3]
            nc.tensor.matmul(out=out_ps[:], lhsT=blob_sb[0:1, ON0:ON0 + BL],
                             rhs=blob_sb[0:1, B0:B0 + C], start=False,
                             stop=True)

            out_sb = cp.tile([BL, C], F32, tag="out_sb")
            nc.vector.tensor_copy(out_sb[:], out_ps[:])
            nc.sync.dma_start(out[:, :], out_sb[:], single_packet=True)

    nc.compile()
    return nc


_CACHE: dict[int, object] = {}


def _get(G: int):
    if G not in _CACHE:
        _CACHE[G] = build(G)
    return _CACHE[G]


def _assign(lens: np.ndarray) -> list[list[int]]:
    """Assign samples to cores, BL per core, minimizing max sum(len)."""
    bins: list[list[int]] = [[] for _ in range(NCORES)]
    loads = np.zeros(NCORES, np.int64)
    for i in np.argsort(-lens, kind="stable"):
        open_ = [c for c in range(NCORES) if len(bins[c]) < BL]
        c = min(open_, key=lambda c: loads[c])
        bins[c].append(int(i))
        loads[c] += lens[i]
    # pairwise-swap local search to shave the max bin
    for _ in range(64):
        hi = int(np.argmax(loads))
        best = None
        for lo in range(NCORES):
            if lo == hi:
                continue
            for a in bins[hi]:
                for bb in bins[lo]:
                    d = lens[a] - lens[bb]
                    if d <= 0:
                        continue
                    new_hi, new_lo = loads[hi] - d, loads[lo] + d
                    peak = max(new_hi, new_lo)
                    if peak < loads[hi] and (best is None or peak < best[0]):
                        best = (peak, lo, a, bb)
        if best is None:
            break
        _, lo, a, bb = best
        bins[hi].remove(a)
        bins[lo].remove(bb)
        bins[hi].append(bb)
        bins[lo].append(a)
        d = lens[a] - lens[bb]
        loads[hi] -= d
        loads[lo] += d
    return bins


def kernel(last_hidden_state, pooled_output, position_indices, W, b):
    lhs = np.ascontiguousarray(last_hidden_state, dtype=np.float32)
    pooled = np.ascontiguousarray(pooled_output, dtype=np.float32)
    pos = np.ascontiguousarray(position_indices, dtype=np.int32)
    W = np.ascontiguousarray(W, dtype=np.float32)
    b = np.ascontiguousarray(b, dtype=np.float32)

    starts = pos[:, 0].astype(np.int64)
    lens = (pos[:, 1] - pos[:, 0]).astype(np.int64)
    bins = _assign(lens)
    maxload = max(int(lens[ids].sum()) for ids in bins)
    G = 1
    while G * P < maxload:
        G *= 2

    WM0, ON0, B0, BW = _cols(G)
    wT = W.reshape(C, KC, P).transpose(2, 1, 0).reshape(P, KC * C)
    in_maps = []
    for ids in bins:
        idx = np.zeros(G * P, np.int32)
        wm = np.zeros((P, BL * G), np.float32)
        r = 0
        for sloc, sid in enumerate(ids):
            L = int(lens[sid])
            rr = np.arange(r, r + L)
            gg, pp = rr // P, rr % P
            idx[gg * P + pp] = sloc * S + int(starts[sid]) + np.arange(L)
            wm[pp, gg * BL + sloc] = 1.0 / L
            r += L
        blob = np.zeros((P, BW), np.float32)
        blob[:, PT0:PT0 + HC * BL] = (
            pooled[ids].reshape(BL, HC, P).transpose(2, 1, 0).reshape(P, -1))
        blob[:, WT0:WT0 + KC * C] = wT
        blob[:, WM0:WM0 + BL * G] = wm
        blob[0, ON0:ON0 + BL] = 1.0
        blob[0, B0:B0 + C] = b
        in_maps.append({
            "lhs": lhs[ids].reshape(BL * S, H),
            "idx": idx.reshape(G, P).T.copy(),
            "blob": blob.astype(ml_dtypes.bfloat16),
        })

    if RUN_KWARGS:
        # profiling path (test.py sets trace=True)
        res = run_bass_kernel_spmd(_get(G), in_maps,
                                   core_ids=list(range(NCORES)),
                                   **RUN_KWARGS)
        global LAST_RESULT
        LAST_RESULT = res
        results = res.results
    else:
        results = _run_fast(G, in_maps)

    out = np.zeros((B, C), np.float32)
    for cid, ids in enumerate(bins):
        out[ids] = results[cid]["out"]
    return out


# Cached-jit fast path: run_bass_kernel_spmd re-jits its PJRT wrapper on
# every call (~17s), so repeated kernel() calls would pay the full XLA +
# neuronx-cc pipeline each time.  This replicates bass2jax.run_bass_via_pjrt
# (multi-core branch) once per G and reuses the compiled executable.
_RUNNER_CACHE: dict = {}


def _get_runner(G):
    if G in _RUNNER_CACHE:
        return _RUNNER_CACHE[G]
    import jax
    from jax.experimental.shard_map import shard_map
    from jax.sharding import Mesh, PartitionSpec
    from concourse import bass2jax

    nc = _get(G)
    bass2jax.install_neuronx_cc_hook()
    assert nc.dbg_addr is None, "fast path assumes debug-free program"
    partition_name = (nc.partition_id_tensor.name
                      if nc.partition_id_tensor else None)

    in_names, out_names, out_avals = [], [], []
    for alloc in nc.m.functions[0].allocations:
        if not isinstance(alloc, mybir.MemoryLocationSet):
            continue
        name = alloc.memorylocations[0].name
        if alloc.kind == "ExternalInput":
            if name != partition_name:
                in_names.append(name)
        elif alloc.kind == "ExternalOutput":
            shape = tuple(alloc.tensor_shape)
            dtype = mybir.dt.np(alloc.dtype)
            out_names.append(name)
            out_avals.append(jax.core.ShapedArray(shape, dtype))
    n_params = len(in_names)
    n_outs = len(out_avals)
    all_names = in_names + out_names
    if partition_name is not None:
        all_names = all_names + [partition_name]

    def _body(*args):
        operands = list(args)
        if partition_name is not None:
            operands.append(bass2jax.partition_id_tensor())
        outs = bass2jax._bass_exec_p.bind(
            *operands,
            out_avals=tuple(out_avals),
            in_names=tuple(all_names),
            out_names=tuple(out_names),
            lowering_input_output_aliases=(),
            sim_require_finite=True,
            sim_require_nnan=True,
            nc=nc,
        )
        return tuple(outs)

    devices = jax.devices()[:NCORES]
    mesh = Mesh(np.asarray(devices), ("core",))
    specs = (PartitionSpec("core"),) * (n_params + n_outs)
    out_specs = (PartitionSpec("core"),) * n_outs
    donate = (tuple(range(n_params, n_params + n_outs))
              if devices[0].platform != "cpu" else ())
    sharded = jax.jit(
        shard_map(_body, mesh=mesh, in_specs=specs, out_specs=out_specs,
                  check_rep=False),
        donate_argnums=donate,
        keep_unused=True,
    )
    runner = (sharded, in_names, out_names, out_avals, n_params)
    _RUNNER_CACHE[G] = runner
    return runner


def _run_fast(G, in_maps):
    sharded, in_names, out_names, out_avals, n_params = _get_runner(G)
    concat_in = [
        np.concatenate([np.asarray(in_maps[c][k]) for c in range(NCORES)],
                       axis=0)
        for k in in_names
    ]
    concat_zeros = [
        np.zeros((NCORES * a.shape[0], *a.shape[1:]), a.dtype)
        for a in out_avals
    ]
    out_arrs = sharded(*concat_in, *concat_zeros)
    return [
        {name: np.asarray(out_arrs[i]).reshape(NCORES, *out_avals[i].shape)[c]
         for i, name in enumerate(out_names)}
        for c in range(NCORES)
    ]


# test/bench hooks (harness just calls kernel(); these stay default)
RUN_KWARGS: dict = {}
LAST_RESULT = None


# revision 13
# speedup vs baseline: 1.3666x; 1.1089x over previous
"""Trainium2 Bass kernel: aspect-level sentiment classification head.

  aspect[b] = mean(last_hidden_state[b, start_b:end_b, :])   (ragged spans)
  out = concat([pooled, aspect], -1) @ W.T + b

Strategy: data-parallel over batch with host-side load balancing.  Samples
are assigned to cores (8 per core) so the per-core total span length is
minimized; each core gathers exactly its spans' rows (tightly packed, no
per-sample padding) from DRAM with G indirect DMAs of 128 rows each.  All
index/mask arithmetic happens on the host: the kernel receives ready-made
gather indices plus a bf16 "weight mask" whose entries are 1/len placed at
(row, sample) positions, so a single bf16 matmul per 128-column hidden chunk
produces the *transposed* aspect features directly.  Pooled features and W
arrive host-pre-transposed in one bf16 blob; the pooled half of the final
GEMM is issued before the gather completes so it overlaps the DMA.
"""

import sys

if "/opt/trn_rl_repo" not in sys.path:
    sys.path.insert(0, "/opt/trn_rl_repo")

import numpy as np
import ml_dtypes

import concourse.tile as tile
from concourse import bacc, mybir
from concourse.bass import IndirectOffsetOnAxis
from concourse.bass_utils import run_bass_kernel_spmd

F32 = mybir.dt.float32
BF16 = mybir.dt.bfloat16
I32 = mybir.dt.int32

B, S, H, C = 64, 4096, 768, 3
NCORES = 8
BL = B // NCORES          # samples per core
P = 128
HC = H // P               # 6 hidden chunks of 128
KC = 2 * H // P           # 12 contraction chunks in the final GEMM

# blob column layout (all bf16): pT | wT | wmask | ones8 | bias
PT0, WT0 = 0, HC * BL                        # 0, 48
def _cols(G):
    wm0 = WT0 + KC * C                       # 84
    on0 = wm0 + BL * G
    b0 = on0 + BL
    return wm0, on0, b0, b0 + C


# partition-id tensor off for HW (saves a ~1.3us register load in the
# preamble); the CPU-sim path needs it on, so sim harnesses set PID = True
PID = False


def build(G: int):
    """Per-core SPMD program gathering G*128 tightly packed span rows."""
    WM0, ON0, B0, BW = _cols(G)
    nc = bacc.Bacc("TRN2", target_bir_lowering=False, debug=False,
                   num_devices=NCORES, enable_partition_id=PID)
    lhs = nc.dram_tensor("lhs", [BL * S, H], F32, kind="ExternalInput").ap()
    idx = nc.dram_tensor("idx", [P, G], I32, kind="ExternalInput").ap()
    blob = nc.dram_tensor("blob", [P, BW], BF16, kind="ExternalInput").ap()
    out = nc.dram_tensor("out", [BL, C], F32, kind="ExternalOutput").ap()

    with tile.TileContext(nc) as tc:
        with (
            tc.tile_pool(name="const", bufs=1) as cp,
            tc.tile_pool(name="rows", bufs=min(2, G)) as rp,
            tc.tile_pool(name="pout", bufs=1, space="PSUM") as po,
            tc.tile_pool(name="pasp", bufs=1, space="PSUM") as pa,
        ):
            idx_sb = cp.tile([P, G], I32, tag="idx_sb")
            nc.sync.dma_start(idx_sb[:], idx[:, :], single_packet=True)
            blob_sb = cp.tile([P, BW], BF16, tag="blob_sb")
            nc.scalar.dma_start(blob_sb[:], blob[:, :])
            pT = blob_sb[:, PT0:PT0 + HC * BL]
            wT = blob_sb[:, WT0:WT0 + KC * C]

            # pooled half of the final GEMM runs while the gather is in
            # flight; the accumulation group stays open until the bias term
            out_ps = po.tile([BL, C], F32, tag="out_ps")
            for c in range(HC):
                nc.tensor.matmul(out=out_ps[:], lhsT=pT[:, c * BL:(c + 1) * BL],
                                 rhs=wT[:, c * C:(c + 1) * C],
                                 start=(c == 0), stop=False)

            # aspT[h, s] = sum_r rows[r, h] * wmask[r, s]
            if G == 1:
                aspT_all = pa.tile([P, HC * BL], F32, tag="aspT")
                aspT_ps = [aspT_all[:, c * BL:(c + 1) * BL] for c in range(HC)]
            else:
                aspT_ps = [pa.tile([P, BL], F32, tag=f"aspT{c}",
                                   name=f"aspT{c}")[:] for c in range(HC)]
            for g in range(G):
                # SWDGE casts f32 -> bf16 during the gather (probed on HW)
                rows_b = rp.tile([P, H], BF16, tag="rows_b")
                nc.gpsimd.indirect_dma_start(
                    out=rows_b[:], out_offset=None, in_=lhs[:, :],
                    in_offset=IndirectOffsetOnAxis(
                        ap=idx_sb[:, g:g + 1], axis=0))
                wm_g = blob_sb[:, WM0 + g * BL:WM0 + (g + 1) * BL]
                for c in range(HC):
                    nc.tensor.matmul(out=aspT_ps[c][:, :],
                                     lhsT=rows_b[:, c * P:(c + 1) * P],
                                     rhs=wm_g, start=(g == 0), stop=(g == G - 1))

            aspT_sb = cp.tile([P, HC * BL], BF16, tag="aspT_sb")
            if G == 1:
                nc.vector.tensor_copy(aspT_sb[:], aspT_all[:])
            else:
                for c in range(HC):
                    nc.vector.tensor_copy(aspT_sb[:, c * BL:(c + 1) * BL],
                                          aspT_ps[c])
            for c in range(HC):
                nc.tensor.matmul(out=out_ps[:],
                                 lhsT=aspT_sb[:, c * BL:(c + 1) * BL],
                                 rhs=wT[:, (HC + c) * C:(HC + c + 1) * C],
                                 start=False, stop=False)
            # bias as a rank-1 accumulation: ones[1,8].T @ bias_row[1,3]
            nc.tensor.matmul(out=out_ps[:], lhsT=blob_sb[0:1, ON0:ON0 + BL],
                             rhs=blob_sb[0:1, B0:B0 + C], start=False,
                             stop=True)

            out_sb = cp.tile([BL, C], F32, tag="out_sb")
            nc.vector.tensor_copy(out_sb[:], out_ps[:])
            nc.sync.dma_start(out[:, :], out_sb[:], single_packet=True)

    nc.compile()
    return nc


_CACHE: dict = {}


def _get(G: int):
    key = (G, PID)
    if key not in _CACHE:
        _CACHE[key] = build(G)
    return _CACHE[key]


def _assign(lens: np.ndarray) -> list[list[int]]:
    """Assign samples to cores, BL per core, minimizing max sum(len)."""
    bins: list[list[int]] = [[] for _ in range(NCORES)]
    loads = np.zeros(NCORES, np.int64)
    for i in np.argsort(-lens, kind="stable"):
        open_ = [c for c in range(NCORES) if len(bins[c]) < BL]
        c = min(open_, key=lambda c: loads[c])
        bins[c].append(int(i))
        loads[c] += lens[i]
    # pairwise-swap local search to shave the max bin
    for _ in range(64):
        hi = int(np.argmax(loads))
        best = None
        for lo in range(NCORES):
            if lo == hi:
                continue
            for a in bins[hi]:
                for bb in bins[lo]:
                    d = lens[a] - lens[bb]
                    if d <= 0:
                        continue
                    new_hi, new_lo = loads[hi] - d, loads[lo] + d
                    peak = max(new_hi, new_lo)
                    if peak < loads[hi] and (best is None or peak < best[0]):
                        best = (peak, lo, a, bb)
        if best is None:
            break
        _, lo, a, bb = best
        bins[hi].remove(a)
        bins[lo].remove(bb)
        bins[hi].append(bb)
        bins[lo].append(a)
        d = lens[a] - lens[bb]
        loads[hi] -= d
        loads[lo] += d
    return bins


def kernel(last_hidden_state, pooled_output, position_indices, W, b):
    lhs = np.ascontiguousarray(last_hidden_state, dtype=np.float32)
    pooled = np.ascontiguousarray(pooled_output, dtype=np.float32)
    pos = np.ascontiguousarray(position_indices, dtype=np.int32)
    W = np.ascontiguousarray(W, dtype=np.float32)
    b = np.ascontiguousarray(b, dtype=np.float32)

    starts = pos[:, 0].astype(np.int64)
    lens = (pos[:, 1] - pos[:, 0]).astype(np.int64)
    bins = _assign(lens)
    maxload = max(int(lens[ids].sum()) for ids in bins)
    G = 1
    while G * P < maxload:
        G *= 2

    WM0, ON0, B0, BW = _cols(G)
    wT = W.reshape(C, KC, P).transpose(2, 1, 0).reshape(P, KC * C)
    in_maps = []
    for ids in bins:
        idx = np.zeros(G * P, np.int32)
        wm = np.zeros((P, BL * G), np.float32)
        r = 0
        for sloc, sid in enumerate(ids):
            L = int(lens[sid])
            rr = np.arange(r, r + L)
            gg, pp = rr // P, rr % P
            idx[gg * P + pp] = sloc * S + int(starts[sid]) + np.arange(L)
            wm[pp, gg * BL + sloc] = 1.0 / L
            r += L
        blob = np.zeros((P, BW), np.float32)
        blob[:, PT0:PT0 + HC * BL] = (
            pooled[ids].reshape(BL, HC, P).transpose(2, 1, 0).reshape(P, -1))
        blob[:, WT0:WT0 + KC * C] = wT
        blob[:, WM0:WM0 + BL * G] = wm
        blob[0, ON0:ON0 + BL] = 1.0
        blob[0, B0:B0 + C] = b
        in_maps.append({
            "lhs": lhs[ids].reshape(BL * S, H),
            "idx": idx.reshape(G, P).T.copy(),
            "blob": blob.astype(ml_dtypes.bfloat16),
        })

    if RUN_KWARGS:
        # profiling path (test.py sets trace=True)
        res = run_bass_kernel_spmd(_get(G), in_maps,
                                   core_ids=list(range(NCORES)),
                                   **RUN_KWARGS)
        global LAST_RESULT
        LAST_RESULT = res
        results = res.results
    else:
        results = _run_fast(G, in_maps)

    out = np.zeros((B, C), np.float32)
    for cid, ids in enumerate(bins):
        out[ids] = results[cid]["out"]
    return out


# Cached-jit fast path: run_bass_kernel_spmd re-jits its PJRT wrapper on
# every call (~17s), so repeated kernel() calls would pay the full XLA +
# neuronx-cc pipeline each time.  This replicates bass2jax.run_bass_via_pjrt
# (multi-core branch) once per G and reuses the compiled executable.
_RUNNER_CACHE: dict = {}


def _get_runner(G):
    if (G, PID) in _RUNNER_CACHE:
        return _RUNNER_CACHE[(G, PID)]
    import jax
    from jax.experimental.shard_map import shard_map
    from jax.sharding import Mesh, PartitionSpec
    from concourse import bass2jax

    nc = _get(G)
    bass2jax.install_neuronx_cc_hook()
    assert nc.dbg_addr is None, "fast path assumes debug-free program"
    partition_name = (nc.partition_id_tensor.name
                      if nc.partition_id_tensor else None)

    in_names, out_names, out_avals = [], [], []
    for alloc in nc.m.functions[0].allocations:
        if not isinstance(alloc, mybir.MemoryLocationSet):
            continue
        name = alloc.memorylocations[0].name
        if alloc.kind == "ExternalInput":
            if name != partition_name:
                in_names.append(name)
        elif alloc.kind == "ExternalOutput":
            shape = tuple(alloc.tensor_shape)
            dtype = mybir.dt.np(alloc.dtype)
            out_names.append(name)
            out_avals.append(jax.core.ShapedArray(shape, dtype))
    n_params = len(in_names)
    n_outs = len(out_avals)
    all_names = in_names + out_names
    if partition_name is not None:
        all_names = all_names + [partition_name]

    def _body(*args):
        operands = list(args)
        if partition_name is not None:
            operands.append(bass2jax.partition_id_tensor())
        outs = bass2jax._bass_exec_p.bind(
            *operands,
            out_avals=tuple(out_avals),
            in_names=tuple(all_names),
            out_names=tuple(out_names),
            lowering_input_output_aliases=(),
            sim_require_finite=True,
            sim_require_nnan=True,
            nc=nc,
        )
        return tuple(outs)

    devices = jax.devices()[:NCORES]
    mesh = Mesh(np.asarray(devices), ("core",))
    specs = (PartitionSpec("core"),) * (n_params + n_outs)
    out_specs = (PartitionSpec("core"),) * n_outs
    donate = (tuple(range(n_params, n_params + n_outs))
              if devices[0].platform != "cpu" else ())
    sharded = jax.jit(
        shard_map(_body, mesh=mesh, in_specs=specs, out_specs=out_specs,
                  check_rep=False),
        donate_argnums=donate,
        keep_unused=True,
    )
    runner = (sharded, in_names, out_names, out_avals, n_params)
    _RUNNER_CACHE[(G, PID)] = runner
    return runner


def _run_fast(G, in_maps):
    sharded, in_names, out_names, out_avals, n_params = _get_runner(G)
    concat_in = [
        np.concatenate([np.asarray(in_maps[c][k]) for c in range(NCORES)],
                       axis=0)
        for k in in_names
    ]
    concat_zeros = [
        np.zeros((NCORES * a.shape[0], *a.shape[1:]), a.dtype)
        for a in out_avals
    ]
    out_arrs = sharded(*concat_in, *concat_zeros)
    return [
        {name: np.asarray(out_arrs[i]).reshape(NCORES, *out_avals[i].shape)[c]
         for i, name in enumerate(out_names)}
        for c in range(NCORES)
    ]


# test/bench hooks (harness just calls kernel(); these stay default)
RUN_KWARGS: dict = {}
LAST_RESULT = None
